# revision 14
# baseline (speedup 1.0000x reference)
"""Trainium2 Bass kernel for nn_AttentionModel_6468220748059.

Self-contained: host-folds BN/conv weights + attention weight stacks,
shards batch B=32 across 8 NeuronCores (4 per core), runs a Tile kernel.

Distance-weighted attention trick: with W=|s-t|/n, c=E^-0.5,
  A[t,s] = sum_d khat[t,d] q[s,d] - k[t,d] qhat[s,d]  (khat=(t/n)k, qhat=(s/n)q)
         = ((t-s)/n) * (q_s . k_t)
so exp(c*W*(q.k)) = exp(+c*A) for t>=s, exp(-c*A) for t<s — exp's per-tile
scale handles the W multiply for free; only diagonal 128x128 blocks need a
DVE sign-fix. Softmax denom comes from an appended ones column in V.
"""

import numpy as np

B, L, S, E, H, NCLS = 32, 4100, 1024, 16, 4, 10
DH = E // H
EPS = 1e-5
CSCALE = float(E) ** -0.5
NB = 4  # batches per core
NCORES = 8
TT = 128

# Schraudolph bf16-bits exp on DVE: i16 = rint(x*A + B); bitcast -> bf16
# approximates 2^(x*log2(e)) with ~3.3% max rel err. Applied only to a
# subset of non-diagonal score tiles to offload the ScalarE exp bottleneck.
LOG2E = 1.4426950408889634
SCH_A = CSCALE * LOG2E * 128.0
SCH_B = 127.0 * 128.0 - 0.044 * 128.0

# Tuning knobs (build-cache keyed on these).
CFG = {
    # (t,hp) pairs per query-half sh handled by DVE-Schraudolph, 0..8
    "n_dve": 4,
    # Newton iterations for the LN inverse-sqrt (2 = exact-ish, 1 = ~0.2% err)
    "rsqrt_iters": 2,
}


# ----------------------------------------------------------------- host prep
def host_prep(inputs):
    f32 = np.float32
    p = {}
    inp = {k: np.asarray(v, dtype=f32) for k, v in inputs.items()}
    bnscale = 1.0 / np.sqrt(1.0 + EPS)

    s1 = inp["bn1_g"] * bnscale
    w1 = inp["patch_w"][:, 0, :] * s1[:, None]
    p["w1T"] = np.ascontiguousarray(w1.T)  # [8k, 8c]
    p["b1"] = (inp["patch_b"] * s1 + inp["bn1_b"]).reshape(8, 1).astype(f32)

    s2 = inp["bn2_g"] * bnscale
    w2 = inp["emb_w"] * s2[:, None, None]
    w2tap = np.zeros((8, 8, 32), f32)
    for k in range(8):
        for ci in range(8):
            w2tap[ci, k, :] = w2[:, ci, k]
    p["w2tap"] = w2tap
    p["b2"] = (inp["emb_b"] * s2 + inp["bn2_b"]).reshape(32, 1).astype(f32)

    s3 = inp["bn3_g"] * bnscale
    dw, pw = inp["dw_w"], inp["pw_w"][:, :, 0]
    comb = np.zeros((32, 32, 16), f32)
    for k in range(32):
        for m in range(16):
            for j in range(2):
                comb[k, 2 * m + j, :] = s3 * pw[:, m] * dw[m, j, k]
    w3T = np.zeros((8, 128, 16), f32)
    for g in range(8):
        for kk in range(4):
            w3T[g, kk * 32 : kk * 32 + 32, :] = comb[4 * g + kk]
    p["w3T"] = np.ascontiguousarray(w3T.transpose(1, 0, 2))  # [128, 8, 16]
    p["b3"] = inp["bn3_b"].reshape(16, 1).astype(f32)

    pos = np.arange(S, dtype=f32)[:, None]
    div = np.exp(np.arange(0, E, 2, dtype=f32) * (-np.log(10000.0) / E))
    ang = pos * div[None, :] * (E / S)
    pe = np.zeros((S, E), f32)
    pe[:, 0::2] = np.sin(ang)
    pe[:, 1::2] = np.cos(ang)
    p["peT"] = np.ascontiguousarray(pe.T)  # [16, S]
    pf = np.zeros((128, 128), f32)
    for j in range(8):
        pf[:, j * 16 : (j + 1) * 16] = pe[128 * j : 128 * (j + 1), :]
    p["pe_fold"] = pf

    for pref in ("1", "2"):
        wq, wk, wv = inp[f"q{pref}_w"], inp[f"k{pref}_w"], inp[f"v{pref}_w"]
        Wq = np.zeros((48, 4, 32), f32)
        Wk = np.zeros((48, 4, 32), f32)
        for h in range(4):
            Wq[0:16, h, 0:4] = wq[4 * h : 4 * h + 4, :].T
            Wq[32:48, h, 4:8] = wq[4 * h : 4 * h + 4, :].T
            Wk[32:48, h, 0:4] = -wk[4 * h : 4 * h + 4, :].T
            Wk[0:16, h, 4:8] = wk[4 * h : 4 * h + 4, :].T
        p[f"Wq{pref}"] = Wq
        p[f"Wk{pref}"] = Wk
        Wv = np.zeros((17, 128), f32)
        for h in range(4):
            Wv[0:16, 32 * h : 32 * h + 4] = wv[4 * h : 4 * h + 4, :].T
            Wv[16, 32 * h + 4] = 1.0
        p[f"Wv{pref}"] = Wv

    p["sgnmask"] = np.sign(
        np.arange(TT, dtype=f32)[:, None] - np.arange(TT, dtype=f32)[None, :]
    ).astype(f32)
    sgnext = -np.ones((128, 512), f32)
    sgnext[:, 0:128] = p["sgnmask"]
    p["sgnext"] = sgnext
    sv = np.zeros((48, S), f32)
    sv[32:48, :] = -(np.arange(S, dtype=f32) / S)[None, :]
    p["svecneg"] = sv
    p["ones_row"] = np.ones((1, S), f32)
    sel = np.zeros((128, 20), f32)
    for h in range(4):
        for j in range(5):
            sel[32 * h + j, 5 * h + j] = 1.0
    p["selT"] = sel
    p["identity"] = np.eye(128, dtype=f32)
    p["identity16b"] = np.eye(16, dtype=f32)
    p["ones_col"] = np.ones((128, 1), f32)
    for nm in ("lna1", "ln1", "lna2", "ln2"):
        p[f"{nm}_g"] = np.broadcast_to(inp[f"{nm}_g"], (128, 16)).copy()
        p[f"{nm}_b"] = np.broadcast_to(inp[f"{nm}_b"], (128, 16)).copy()
    selE = np.zeros((128, 16), f32)
    for j in range(8):
        for e in range(16):
            selE[16 * j + e, e] = 1.0 / S
    p["selE"] = selE
    p["woutT"] = np.ascontiguousarray(inp["out_w"].T)
    p["bout"] = inp["out_b"].reshape(1, NCLS).astype(f32)

    x = inp["x"][:, 0, :]
    x8 = np.zeros((B, 8, S), f32)
    for k in range(8):
        x8[:, k, :] = x[:, k : k + 4 * S : 4][:, :S]
    p["x8"] = x8

    # which LN affine transforms are trivial (skip ops)
    p["_ln_trivial"] = {
        nm: bool(
            np.allclose(inp[f"{nm}_g"], 1.0) and np.allclose(inp[f"{nm}_b"], 0.0)
        )
        for nm in ("lna1", "ln1", "lna2", "ln2")
    }
    return p


# ------------------------------------------------------------- kernel build
_BUILD_CACHE = {}

CONST_SPECS = [
    # name, shape, dtype tag: False=f32, True=bf16, "r"=float32r
    ("w1T", (8, 8), True),
    ("b1", (8, 1), False),
    ("w2tap", (8, 8, 32), True),
    ("b2", (32, 1), False),
    ("w3T", (128, 8, 16), True),
    ("b3", (16, 1), False),
    ("peT", (16, S), True),
    ("pe_fold", (128, 128), False),
    ("Wq1", (48, 4, 32), True),
    ("Wk1", (48, 4, 32), True),
    ("Wv1", (17, 128), True),
    ("Wq2", (48, 4, 32), True),
    ("Wk2", (48, 4, 32), True),
    ("Wv2", (17, 128), True),
    ("sgnmask", (128, 128), False),
    ("sgnext", (128, 512), False),
    ("svecneg", (48, S), True),
    ("ones_row", (1, S), True),
    ("selT", (128, 20), False),
    ("identity", (128, 128), False),
    ("identity16b", (16, 16), True),
    ("ones_col", (128, 1), False),
    ("lna1_g", (128, 16), False),
    ("lna1_b", (128, 16), False),
    ("ln1_g", (128, 16), False),
    ("ln1_b", (128, 16), False),
    ("lna2_g", (128, 16), False),
    ("lna2_b", (128, 16), False),
    ("ln2_g", (128, 16), False),
    ("ln2_b", (128, 16), False),
    ("selE", (128, 16), False),
    ("woutT", (16, NCLS), False),
    ("bout", (1, NCLS), False),
]


def _brd(ap, count):
    """Append a broadcast (step 0) innermost free dim to an AP."""
    import concourse.bass as bass

    return bass.AP(tensor=ap.tensor, offset=ap.offset, ap=[*list(ap.ap), [0, count]])


def build_nc(ln_trivial, reps=1, cfg=None):
    import concourse.bass as bass
    import concourse.bacc as bacc
    import concourse.tile as tile
    from concourse import mybir

    if cfg is None:
        cfg = CFG
    n_dve = cfg["n_dve"]
    rsqrt_iters = cfg["rsqrt_iters"]

    f32 = mybir.dt.float32
    f32r = mybir.dt.float32r
    bf16 = mybir.dt.bfloat16
    i16 = mybir.dt.int16
    ALU = mybir.AluOpType
    AF = mybir.ActivationFunctionType

    # (t, hp) pairs per sh assigned to the DVE-Schraudolph exp, spread
    # across t so ScalarE and DVE run concurrently.
    def dve_pairs_for(sh):
        ts = [t for t in range(8) if t // 4 != sh]
        order = [
            (ts[0], 0), (ts[1], 1), (ts[2], 0), (ts[3], 1),
            (ts[0], 1), (ts[1], 0), (ts[2], 1), (ts[3], 0),
        ]
        return set(order[:n_dve])

    DVE_PAIRS = [dve_pairs_for(0), dve_pairs_for(1)]

    nc = bacc.Bacc(trn_type="TRN2", target_bir_lowering=False, debug=False)

    dram = {}
    dt_of = {False: f32, True: bf16, "r": f32r}
    for name, shape, isbf in CONST_SPECS:
        dram[name] = nc.dram_tensor(
            name, list(shape), dt_of[isbf], kind="ExternalInput"
        ).ap()
    dram["x8"] = nc.dram_tensor("x8", [NB, 8, S], bf16, kind="ExternalInput").ap()
    out_d = nc.dram_tensor("out", [NB, NCLS], f32, kind="ExternalOutput").ap()

    with tile.TileContext(nc) as tc:
        import contextlib

        ctx = contextlib.ExitStack()
        cpool = ctx.enter_context(tc.tile_pool(name="consts", bufs=1))
        perb = ctx.enter_context(tc.tile_pool(name="perb", bufs=NB))
        work = ctx.enter_context(tc.tile_pool(name="work", bufs=4))
        wexp = ctx.enter_context(tc.tile_pool(name="wexp", bufs=6))
        small = ctx.enter_context(tc.tile_pool(name="small", bufs=8))
        pp_s = ctx.enter_context(tc.tile_pool(name="pp_s", bufs=3, space="PSUM"))
        pp_av = ctx.enter_context(tc.tile_pool(name="pp_av", bufs=1, space="PSUM"))
        pp_sm = ctx.enter_context(tc.tile_pool(name="pp_sm", bufs=1, space="PSUM"))

        # ---- constants: critical subset DMA'd now, the rest later
        EARLY = {
            "w1T", "b1", "w2tap", "b2", "w3T", "b3", "identity",
            "identity16b", "peT",
            "ones_row", "svecneg", "Wq1", "Wk1", "Wv1",
            "sgnmask", "sgnext", "selT",
        }
        C = {}
        _late_consts = []
        # conv-critical consts first so batch 0's conv chain starts ASAP;
        # remaining EARLY consts queue behind it, LATE ones after conv(2).
        _prio = ["w1T", "b1", "w2tap", "b2", "w3T", "b3", "identity"]
        _order = _prio + [
            n for n, _, _ in CONST_SPECS if n not in _prio
        ]
        _by_name = {n: (n, s, d) for n, s, d in CONST_SPECS}
        for name in _order:
            name, shape, isbf = _by_name[name]
            t = cpool.tile(list(shape), dt_of[isbf], name=f"c_{name}")
            C[name] = t
            if name not in EARLY:
                _late_consts.append(name)
        for name in _prio:
            nc.sync.dma_start(out=C[name], in_=dram[name])

        def load_late_consts():
            for name in _late_consts:
                nc.sync.dma_start(out=C[name], in_=dram[name])

        h4_b = []        # [16, S] f32 per batch (x_srcT)
        xsrc_fold_b = [] # [128, 128] f32
        att_fold_b = [None] * NB

        eps_sb = cpool.tile([128, 1], f32, name="eps_sb")
        nc.vector.memset(eps_sb, EPS)
        magic_sb = cpool.tile([128, 1], mybir.dt.int32, name="magic_sb")
        nc.vector.memset(magic_sb, 0x5F3759DF)

        def ln_fold(src, nm, extra_add=None):
            return ln_fold_multi([(src, nm, extra_add)])[0]

        def ln_fold_multi(items):
            """items: list of (src, nm, extra_add). Ops are emitted
            interleaved across items so independent chains pipeline on DVE."""
            outs = [None] * len(items)
            st = [dict() for _ in items]
            for i, (src, nm, extra_add) in enumerate(items):
                _ln_stage0(st[i], src, f"{nm}{i}", extra_add)
            for step in range(1, 12):
                for i, (src, nm, extra_add) in enumerate(items):
                    _ln_step(st[i], step, f"{items[i][1]}{i}")
            for i, (src, nm, extra_add) in enumerate(items):
                outs[i] = _ln_final(st[i], f"{nm}{i}", nm)
            return outs

        def _ln_stage0(S_, src, nm, extra_add):
            """LN over e-groups of 16 in folded [128, (j,16)] layout.
            src: [128,128] f32 SBUF tile. Returns new [128,128] f32 tile.
            extra_add: optional [128,128] tile added BEFORE the LN (residual)."""
            if extra_add is not None:
                tmp = small.tile([128, 128], f32, name=f"res_{nm}", tag="lnres")
                nc.vector.tensor_add(tmp, src, extra_add)
                src = tmp
            S_["s3d"] = src.rearrange("p (j e) -> p j e", e=16)

        def _ln_step(S_, step, nm):
            i32 = mybir.dt.int32
            if step == 1:
                S_["sums"] = small.tile([128, 8], f32, name=f"sums_{nm}", tag="lnsum")
                nc.vector.tensor_reduce(
                    out=S_["sums"], in_=S_["s3d"], axis=mybir.AxisListType.X,
                    op=ALU.add,
                )
            elif step == 2:
                S_["negmean"] = small.tile([128, 8], f32, name=f"nm_{nm}", tag="lnnm")
                nc.vector.tensor_scalar_mul(S_["negmean"], S_["sums"], -1.0 / 16.0)
            elif step == 3:
                S_["cen"] = small.tile([128, 8, 16], f32, name=f"cen_{nm}", tag="lncen")
                nc.vector.tensor_tensor(
                    out=S_["cen"], in0=S_["s3d"], in1=_brd(S_["negmean"], 16),
                    op=ALU.add,
                )
            elif step == 4:
                S_["sq"] = small.tile([128, 8, 16], f32, name=f"sq_{nm}", tag="lnsq")
                nc.vector.tensor_mul(S_["sq"], S_["cen"], S_["cen"])
            elif step == 5:
                var = small.tile([128, 8], f32, name=f"var_{nm}", tag="lnvar")
                nc.vector.tensor_reduce(
                    out=var, in_=S_["sq"], axis=mybir.AxisListType.X, op=ALU.add
                )
                S_["var"] = var
            elif step == 6:
                v16 = small.tile([128, 8], f32, name=f"v16_{nm}", tag="lnv16")
                nc.vector.tensor_scalar(
                    out=v16, in0=S_["var"], scalar1=1.0 / 16.0, scalar2=EPS,
                    op0=ALU.mult, op1=ALU.add,
                )
                S_["v16"] = v16
                ish = small.tile([128, 8], i32, name=f"ish_{nm}", tag="lnish")
                nc.vector.tensor_scalar(
                    out=ish, in0=v16.bitcast(i32), scalar1=1, scalar2=None,
                    op0=ALU.arith_shift_right,
                )
                S_["ish"] = ish
            elif step == 7:
                y = small.tile([128, 8], i32, name=f"y0_{nm}", tag="lny0")
                nc.vector.tensor_tensor(
                    out=y, in0=_brd(magic_sb, 8), in1=S_["ish"], op=ALU.subtract
                )
                S_["y"] = y.bitcast(f32)
            elif step in (8, 9):
                it = step - 8
                if it >= rsqrt_iters:
                    return
                y = S_["y"]
                t1 = small.tile([128, 8], f32, name=f"nt1_{nm}_{it}", tag="lnnt1")
                nc.vector.tensor_mul(t1, y, y)
                t2 = small.tile([128, 8], f32, name=f"nt2_{nm}_{it}", tag="lnnt2")
                nc.vector.tensor_mul(t2, t1, S_["v16"])
                t3 = small.tile([128, 8], f32, name=f"nt3_{nm}_{it}", tag="lnnt3")
                nc.vector.tensor_scalar(
                    out=t3, in0=t2, scalar1=-0.5, scalar2=1.5,
                    op0=ALU.mult, op1=ALU.add,
                )
                yn = small.tile([128, 8], f32, name=f"yn_{nm}_{it}", tag="lnyn")
                nc.vector.tensor_mul(yn, y, t3)
                S_["y"] = yn
            elif step == 10:
                dst = small.tile([128, 8, 16], f32, name=f"ln_{nm}", tag="lnout")
                nc.vector.tensor_tensor(
                    out=dst, in0=S_["cen"], in1=_brd(S_["y"], 16), op=ALU.mult
                )
                S_["dst"] = dst

        def _ln_final(S_, nm, base_nm):
            dst = S_["dst"]
            dst2 = dst.rearrange("p j e -> p (j e)")
            base = base_nm[:-2] if base_nm.endswith("_l") else base_nm
            if not ln_trivial[base]:
                g3 = C[f"{base}_g"].rearrange("p e -> p 1 e")
                b3 = C[f"{base}_b"].rearrange("p e -> p 1 e")
                dstg = small.tile([128, 8, 16], f32, name=f"lng_{nm}", tag="lnoutg")
                nc.vector.tensor_tensor(
                    out=dstg,
                    in0=dst,
                    in1=bass.AP(
                        tensor=g3.tensor, offset=g3.offset,
                        ap=[g3.ap[0], [0, 8], g3.ap[2]],
                    ),
                    op=ALU.mult,
                )
                dstb = small.tile([128, 8, 16], f32, name=f"lnb_{nm}", tag="lnoutb")
                nc.vector.tensor_tensor(
                    out=dstb,
                    in0=dstg,
                    in1=bass.AP(
                        tensor=b3.tensor, offset=b3.offset,
                        ap=[b3.ap[0], [0, 8], b3.ap[2]],
                    ),
                    op=ALU.add,
                )
                dst2 = dstb.rearrange("p j e -> p (j e)")
            return dst2

        # ================= pipeline =================
        h4_b = {}
        xsrc_fold_b = {}
        P = {}  # (li, b) -> dict of prep tiles
        O = {}  # (li, b) -> o_sb/o_ps refs

        x8_tiles = {}

        def load_x8(b):
            x8 = work.tile([8, S], bf16, name=f"x8_{b}", tag="x8")
            nc.sync.dma_start(out=x8, in_=dram["x8"][b])
            x8_tiles[b] = x8

        def conv_frontend(b):
            x8 = x8_tiles[b]
            h1p = work.tile([8, S + 8], bf16, name=f"h1p_{b}", tag="h1p")
            nc.vector.memset(h1p[:, 0:3], 0.0)
            nc.vector.memset(h1p[:, 3 + S :], 0.0)
            for c in range(2):
                ps = pp_sm.tile([8, 512], f32, name=f"h1ps_{b}_{c}", tag="psmall")
                nc.tensor.matmul(
                    ps, C["w1T"], x8[:, 512 * c : 512 * (c + 1)],
                    start=True, stop=True,
                )
                nc.vector.tensor_scalar(
                    out=h1p[:, 3 + 512 * c : 3 + 512 * (c + 1)], in0=ps,
                    scalar1=C["b1"], scalar2=0.0, op0=ALU.add, op1=ALU.max,
                )
            h2p = work.tile([32, S + 36], bf16, name=f"h2p_{b}", tag="h2p")
            nc.vector.memset(h2p[:, 0:15], 0.0)
            nc.vector.memset(h2p[:, 15 + S :], 0.0)
            for c in range(2):
                ps = pp_sm.tile([32, 512], f32, name=f"h2ps_{b}_{c}", tag="psmall")
                for k in range(8):
                    nc.tensor.matmul(
                        ps, C["w2tap"][:, k, :],
                        h1p[:, k + 512 * c : k + 512 * c + 512],
                        start=(k == 0), stop=(k == 7),
                    )
                nc.vector.tensor_scalar(
                    out=h2p[:, 15 + 512 * c : 15 + 512 * (c + 1)], in0=ps,
                    scalar1=C["b2"], scalar2=0.0, op0=ALU.add, op1=ALU.max,
                )
            h2im = work.tile([128, S + 36], bf16, name=f"h2im_{b}", tag="h2im")
            for kk in range(4):
                nc.sync.dma_start(
                    out=h2im[32 * kk : 32 * kk + 32, 0 : S + 32],
                    in_=h2p[:, kk : kk + S + 32],
                )
            h4 = perb.tile([16, S], bf16, name=f"h4_{b}", tag="h4")
            for c in range(2):
                ps = pp_sm.tile([16, 512], f32, name=f"h3ps_{b}_{c}", tag="psmall")
                for g in range(8):
                    nc.tensor.matmul(
                        ps, C["w3T"][:, g, :],
                        h2im[:, 4 * g + 512 * c : 4 * g + 512 * c + 512],
                        start=(g == 0), stop=(g == 7),
                    )
                nc.vector.tensor_scalar(
                    out=h4[:, 512 * c : 512 * (c + 1)], in0=ps,
                    scalar1=C["b3"], scalar2=0.0, op0=ALU.add, op1=ALU.max,
                )
            h4_b[b] = h4
            xs_ps = pp_sm.tile([128, 128], bf16, name=f"xsps_{b}", tag="psmall")
            for j in range(8):
                nc.tensor.transpose(
                    xs_ps[:, 16 * j : 16 * j + 16],
                    h4[:, 128 * j : 128 * (j + 1)],
                    C["identity16b"],
                )
            xsf = perb.tile([128, 128], f32, name=f"xsf_{b}", tag="xsf")
            nc.vector.tensor_copy(xsf, xs_ps)
            xsrc_fold_b[b] = xsf

        def attn_prep(li, pref, b):
            # xx: rows 0-15 xT, 16 ones, 17-31 zero, 32-47 xhatneg=-(s/n)*xT
            xx = work.tile([48, S], bf16, name=f"xx_{li}_{b}", tag="xx")
            nc.gpsimd.memset(xx[0:48, :], 0.0)
            if li == 0:
                nc.vector.tensor_add(xx[0:16, :], h4_b[b], C["peT"])
            else:
                x2f = small.tile([128, 128], f32, name=f"x2f_{b}", tag="x2f")
                nc.vector.tensor_add(x2f, att_fold_b[b], C["pe_fold"])
                for half in range(2):
                    t2 = pp_sm.tile(
                        [16, 512], f32, name=f"t2_{b}_{half}", tag="psmall"
                    )
                    for j in range(4):
                        jj = 4 * half + j
                        nc.tensor.transpose(
                            t2[:, 128 * j : 128 * (j + 1)],
                            x2f[:, 16 * jj : 16 * jj + 16],
                            C["identity"],
                        )
                    nc.vector.tensor_copy(
                        xx[0:16, 512 * half : 512 * (half + 1)], t2
                    )
            nc.gpsimd.dma_start(out=xx[16:17, :], in_=C["ones_row"])
            nc.gpsimd.dma_start(out=xx[32:48, :], in_=xx[0:16, :])
            nc.vector.tensor_mul(
                xx[32:48, :], xx[32:48, :], C["svecneg"][32:48, :]
            )
            qT = work.tile([128, S], bf16, name=f"qT_{li}_{b}", tag="qT")
            kT = work.tile([128, S], bf16, name=f"kT_{li}_{b}", tag="kT")
            for dst, wname in ((qT, f"Wq{pref}"), (kT, f"Wk{pref}")):
                for c in range(2):
                    ps = pp_sm.tile(
                        [128, 512], f32, name=f"qk_{li}_{b}_{c}", tag="psmall"
                    )
                    for h in range(4):
                        nc.tensor.matmul(
                            ps[32 * h : 32 * h + 32, :],
                            C[wname][:, h, :],
                            xx[:, 512 * c : 512 * (c + 1)],
                            start=True, stop=True, tile_position=(0, 32 * h),
                        )
                    nc.vector.tensor_copy(dst[:, 512 * c : 512 * (c + 1)], ps)
            v_sb = work.tile([128, 8, 128], bf16, name=f"v_{li}_{b}", tag="v")
            for g in range(2):
                vps4 = pp_sm.tile(
                    [128, 4, 128], f32, name=f"vps_{li}_{b}_{g}", tag="psmall"
                )
                for tt in range(4):
                    t = 4 * g + tt
                    nc.tensor.matmul(
                        vps4[:, tt, :], xx[0:17, 128 * t : 128 * (t + 1)],
                        C[f"Wv{pref}"], start=True, stop=True,
                    )
                nc.vector.tensor_copy(v_sb[:, 4 * g : 4 * g + 4, :], vps4)
            P[(li, b)] = dict(qT=qT, kT=kT, v_sb=v_sb)

        def attn_core_pair(li, pref, pair):
            o_sb = {}
            for b in pair:
                o_sb[b] = work.tile([128, S], f32, name=f"osb_{li}_{b}", tag="osb")
            for sh in range(2):
                o_ps = {}
                for b in pair:
                    o_ps[b] = pp_av.tile(
                        [128, 512], f32, name=f"ops_{li}_{b}_{sh}", tag="ops"
                    )
                for t in range(8):
                    for hp in range(2):
                        for b in pair:
                            qT, kT, v_sb = (
                                P[(li, b)]["qT"], P[(li, b)]["kT"], P[(li, b)]["v_sb"]
                            )
                            sps = pp_s.tile(
                                [128, 2, 512], f32,
                                name=f"sps_{li}_{b}_{sh}_{t}_{hp}", tag="sps",
                            )
                            for hh in range(2):
                                h = 2 * hp + hh
                                nc.tensor.matmul(
                                    sps[:, hh, :],
                                    kT[32 * h : 32 * h + 8, 128 * t : 128 * (t + 1)],
                                    qT[32 * h : 32 * h + 8, 512 * sh : 512 * (sh + 1)],
                                    start=True, stop=True,
                                    tile_position=(32 * h, 0),
                                )
                            aT = wexp.tile(
                                [128, 2, 512], bf16,
                                name=f"aT_{li}_{b}_{sh}_{t}_{hp}", tag="aT",
                            )
                            if t // 4 != sh and (t, hp) in DVE_PAIRS[sh]:
                                # DVE Schraudolph exp-bits (offloads ScalarE)
                                sgn = 1.0 if t // 4 > sh else -1.0
                                nc.vector.tensor_scalar(
                                    out=aT.bitcast(i16), in0=sps,
                                    scalar1=sgn * SCH_A, scalar2=SCH_B,
                                    op0=ALU.mult, op1=ALU.add,
                                )
                            elif t // 4 == sh:
                                lc = 128 * t - 512 * sh
                                if lc >= 256:
                                    # cheap extended sign fix -> 1 exp call
                                    sgn = C["sgnext"]
                                    nc.vector.tensor_tensor(
                                        out=sps[:, :, lc:512],
                                        in0=sps[:, :, lc:512],
                                        in1=bass.AP(
                                            tensor=sgn.tensor, offset=sgn.offset,
                                            ap=[sgn.ap[0], [0, 2], [1, 512 - lc]],
                                        ),
                                        op=ALU.mult,
                                    )
                                    nc.scalar.activation(
                                        out=aT, in_=sps, func=AF.Exp, scale=CSCALE
                                    )
                                else:
                                    # classic diag fix + split exp
                                    sgn = C["sgnmask"]
                                    nc.vector.tensor_tensor(
                                        out=sps[:, :, lc : lc + 128],
                                        in0=sps[:, :, lc : lc + 128],
                                        in1=bass.AP(
                                            tensor=sgn.tensor, offset=sgn.offset,
                                            ap=[sgn.ap[0], [0, 2], sgn.ap[1]],
                                        ),
                                        op=ALU.mult,
                                    )
                                    bnd = lc + 128
                                    nc.scalar.activation(
                                        out=aT[:, :, 0:bnd], in_=sps[:, :, 0:bnd],
                                        func=AF.Exp, scale=CSCALE,
                                    )
                                    nc.scalar.activation(
                                        out=aT[:, :, bnd:512], in_=sps[:, :, bnd:512],
                                        func=AF.Exp, scale=-CSCALE,
                                    )
                            else:
                                scl = CSCALE if t // 4 > sh else -CSCALE
                                nc.scalar.activation(
                                    out=aT, in_=sps, func=AF.Exp, scale=scl
                                )
                            for hh in range(2):
                                h = 2 * hp + hh
                                nc.tensor.matmul(
                                    o_ps[b][32 * h : 32 * h + 32, :],
                                    v_sb[:, t, 32 * h : 32 * h + 32],
                                    aT[:, hh, :],
                                    start=(t == 0), stop=(t == 7),
                                    tile_position=(0, 32 * h),
                                    skip_group_check=True,
                                )
                for b in pair:
                    nc.vector.tensor_copy(
                        o_sb[b][:, 512 * sh : 512 * (sh + 1)], o_ps[b]
                    )
            for b in pair:
                O[(li, b)] = o_sb[b]

        def attn_epilogue(li, pref, b):
            o_sb = O[(li, b)]
            T_ps = pp_sm.tile([128, 8, 20], f32, name=f"Tps_{li}_{b}", tag="psmall")
            for j in range(8):
                nc.tensor.matmul(
                    T_ps[:, j, :], o_sb[:, 128 * j : 128 * (j + 1)], C["selT"],
                    start=True, stop=True,
                )
            T4 = T_ps.rearrange("p j (h c) -> p j h c", h=4)
            r_sb = small.tile([128, 8, 4], f32, name=f"r_{li}_{b}", tag="recip")
            nc.vector.reciprocal(out=r_sb, in_=T4[:, :, :, 4])
            araw = small.tile([128, 8, 4, 4], f32, name=f"araw_{li}_{b}", tag="araw")
            nc.vector.tensor_tensor(
                out=araw, in0=T4[:, :, :, 0:4], in1=_brd(r_sb, 4), op=ALU.mult
            )
            araw2 = araw.rearrange("p j h c -> p (j h c)")
            return araw2

        def attn_ln_stage(li, pref, pair, araws):
            anorms = ln_fold_multi(
                [(a, f"lna{pref}_l", None) for a in araws]
            )
            if li == 0:
                att1s = ln_fold_multi(
                    [
                        (anorms[i], "ln1_l", xsrc_fold_b[b])
                        for i, b in enumerate(pair)
                    ]
                )
                for i, b in enumerate(pair):
                    att1p = perb.tile([128, 128], f32, name=f"att1_{b}", tag="attf")
                    nc.vector.tensor_copy(att1p, att1s[i])
                    att_fold_b[b] = att1p
                return
            att2s = ln_fold_multi([(a, "ln2_l", None) for a in anorms])
            for i, b in enumerate(pair):
                att2 = att2s[i]
                att2p = small.tile([128, 128], f32, name=f"att2_{b}", tag="att2")
                nc.vector.tensor_copy(att2p, att2)
                cs_ps = pp_sm.tile([128, 1], f32, name=f"cs_{b}", tag="psmall")
                nc.tensor.matmul(cs_ps, att2p, C["ones_col"], start=True, stop=True)
                cs_sb = small.tile([128, 1], f32, name=f"cssb_{b}", tag="cssb")
                nc.vector.tensor_copy(cs_sb, cs_ps)
                pl_ps = pp_sm.tile([16, 1], f32, name=f"pl_{b}", tag="psmall")
                nc.tensor.matmul(pl_ps, C["selE"], cs_sb, start=True, stop=True)
                pl_sb = small.tile([16, 1], f32, name=f"plsb_{b}", tag="plsb")
                nc.vector.tensor_copy(pl_sb, pl_ps)
                o10_ps = pp_sm.tile([1, NCLS], f32, name=f"o10_{b}", tag="psmall")
                nc.tensor.matmul(o10_ps, pl_sb, C["woutT"], start=True, stop=True)
                o10 = small.tile([1, NCLS], f32, name=f"o10sb_{b}", tag="o10")
                nc.vector.tensor_add(o10, o10_ps, C["bout"])
                nc.sync.dma_start(out=out_d[b], in_=o10)

        # ---- software-pipelined emission
        pairs = [(0, 1), (2, 3)]

        def _epi(li, pref, pair):
            araws = [attn_epilogue(li, pref, b) for b in pair]
            attn_ln_stage(li, pref, pair, araws)

        def _emit_pipeline(first):
            load_x8(0)
            for name in sorted(EARLY - set(_prio)):
                nc.sync.dma_start(out=C[name], in_=dram[name])
            for b in range(1, NB):
                load_x8(b)
            conv_frontend(0)
            attn_prep(0, "1", 0)
            attn_core_pair(0, "1", (0,))
            conv_frontend(1)
            attn_prep(0, "1", 1)
            attn_core_pair(0, "1", (1,))
            conv_frontend(2)
            attn_prep(0, "1", 2)
            if first:
                load_late_consts()
            _epi(0, "1", (0,))
            attn_core_pair(0, "1", (2,))
            conv_frontend(3)
            attn_prep(0, "1", 3)
            _epi(0, "1", (1,))
            attn_core_pair(0, "1", (3,))
            attn_prep(1, "2", 0)
            _epi(0, "1", (2,))
            attn_core_pair(1, "2", (0,))
            attn_prep(1, "2", 1)
            _epi(0, "1", (3,))
            attn_core_pair(1, "2", (1,))
            attn_prep(1, "2", 2)
            _epi(1, "2", (0,))
            attn_core_pair(1, "2", (2,))
            attn_prep(1, "2", 3)
            _epi(1, "2", (1,))
            _epi(1, "2", (2,))
            attn_core_pair(1, "2", (3,))
            _epi(1, "2", (3,))

        for _rep in range(reps):
            _emit_pipeline(_rep == 0)
        ctx.close()

    nc.compile()
    return nc


def _get_nc(ln_trivial_key, reps=1, cfg=None):
    if cfg is None:
        cfg = CFG
    key = (
        tuple(sorted(ln_trivial_key.items())),
        reps,
        tuple(sorted(cfg.items())),
    )
    if key not in _BUILD_CACHE:
        _BUILD_CACHE[key] = build_nc(ln_trivial_key, reps, cfg)
    return _BUILD_CACHE[key]


# ------------------------------------------------------------------ runner
def _run(inputs, trace=False, **kw):
    import ml_dtypes
    from concourse import bass_utils

    p = host_prep(inputs)
    nc = _get_nc(p["_ln_trivial"])

    base = {}
    for name, shape, isbf in CONST_SPECS:
        a = p[name].astype(ml_dtypes.bfloat16 if isbf is True else np.float32)
        assert a.shape == shape, (name, a.shape, shape)
        base[name] = a
    in_maps = []
    for c in range(NCORES):
        m = dict(base)
        m["x8"] = np.ascontiguousarray(p["x8"][NB * c : NB * (c + 1)]).astype(ml_dtypes.bfloat16)
        in_maps.append(m)

    res = bass_utils.run_bass_kernel_spmd(
        nc, in_maps, core_ids=list(range(NCORES)), trace=trace, **kw
    )
    out = np.concatenate(
        [res.results[c]["out"] for c in range(NCORES)], axis=0
    ).astype(np.float32)
    return out, res


def kernel(**inputs) -> np.ndarray:
    out, _ = _run(inputs, trace=False)
    return out


if __name__ == "__main__":
    # standalone smoke test (requires reference.py + cached inputs)
    import os

    if os.path.exists("/tmp/inputs.npz"):
        inputs = dict(np.load("/tmp/inputs.npz"))
    else:
        import reference

        inputs = {k: np.asarray(v) for k, v in reference.setup_inputs().items()}
    out = kernel(**inputs)
    print("out shape:", out.shape, "out[0]:", out[0])



# revision 15
# speedup vs baseline: 6.7069x; 6.7069x over previous
"""Trainium2 Bass kernel for nn_AttentionModel_6468220748059.

Self-contained: host-folds BN/conv weights + attention weight stacks,
shards batch B=32 across 8 NeuronCores (4 per core), runs a Tile kernel.

Distance-weighted attention trick: with W=|s-t|/n, c=E^-0.5,
  A[t,s] = sum_d khat[t,d] q[s,d] - k[t,d] qhat[s,d]  (khat=(t/n)k, qhat=(s/n)q)
         = ((t-s)/n) * (q_s . k_t)
so exp(c*W*(q.k)) = exp(+c*A) for t>=s, exp(-c*A) for t<s — exp's per-tile
scale handles the W multiply for free; only diagonal 128x128 blocks need a
DVE sign-fix. Softmax denom comes from an appended ones column in V.
"""

import numpy as np

B, L, S, E, H, NCLS = 32, 4100, 1024, 16, 4, 10
DH = E // H
EPS = 1e-5
CSCALE = float(E) ** -0.5
NB = 4  # batches per core
NCORES = 8
TT = 128

# Schraudolph bf16-bits exp on DVE: i16 = rint(x*A + B); bitcast -> bf16
# approximates 2^(x*log2(e)) with ~3.3% max rel err. Applied only to a
# subset of non-diagonal score tiles to offload the ScalarE exp bottleneck.
LOG2E = 1.4426950408889634
SCH_A = CSCALE * LOG2E * 128.0
SCH_B = 127.0 * 128.0 - 0.044 * 128.0

# Tuning knobs (build-cache keyed on these).
CFG = {
    # (t,hp) pairs per query-half sh handled by DVE-Schraudolph, 0..8
    "n_dve": 4,
    # Newton iterations for the LN inverse-sqrt (2 = exact-ish, 1 = ~0.2% err)
    "rsqrt_iters": 2,
    # lc=256 diag tiles: extended sign-mask + 1 exp call (True) vs minimal
    # 128-col mask + split exp (False -> less DVE, more ScalarE)
    "diag_cheap": True,
    # run the LN chain on GpSimd instead of DVE
    "ln_on_pool": False,
    # how many of the 4 qk PSUM->SBUF copies per prep go to ScalarE (0..4)
    "kq_on_act": 0,
    # o_ps -> o_sb copies on ScalarE instead of DVE
    "osb_on_act": False,
}


# ----------------------------------------------------------------- host prep
def host_prep(inputs):
    f32 = np.float32
    p = {}
    inp = {k: np.asarray(v, dtype=f32) for k, v in inputs.items()}
    bnscale = 1.0 / np.sqrt(1.0 + EPS)

    s1 = inp["bn1_g"] * bnscale
    w1 = inp["patch_w"][:, 0, :] * s1[:, None]
    p["w1T"] = np.ascontiguousarray(w1.T)  # [8k, 8c]
    p["b1"] = (inp["patch_b"] * s1 + inp["bn1_b"]).reshape(8, 1).astype(f32)

    s2 = inp["bn2_g"] * bnscale
    w2 = inp["emb_w"] * s2[:, None, None]
    w2tap = np.zeros((8, 8, 32), f32)
    for k in range(8):
        for ci in range(8):
            w2tap[ci, k, :] = w2[:, ci, k]
    p["w2tap"] = w2tap
    p["b2"] = (inp["emb_b"] * s2 + inp["bn2_b"]).reshape(32, 1).astype(f32)

    s3 = inp["bn3_g"] * bnscale
    dw, pw = inp["dw_w"], inp["pw_w"][:, :, 0]
    comb = np.zeros((32, 32, 16), f32)
    for k in range(32):
        for m in range(16):
            for j in range(2):
                comb[k, 2 * m + j, :] = s3 * pw[:, m] * dw[m, j, k]
    w3T = np.zeros((8, 128, 16), f32)
    for g in range(8):
        for kk in range(4):
            w3T[g, kk * 32 : kk * 32 + 32, :] = comb[4 * g + kk]
    p["w3T"] = np.ascontiguousarray(w3T.transpose(1, 0, 2))  # [128, 8, 16]
    p["b3"] = inp["bn3_b"].reshape(16, 1).astype(f32)

    pos = np.arange(S, dtype=f32)[:, None]
    div = np.exp(np.arange(0, E, 2, dtype=f32) * (-np.log(10000.0) / E))
    ang = pos * div[None, :] * (E / S)
    pe = np.zeros((S, E), f32)
    pe[:, 0::2] = np.sin(ang)
    pe[:, 1::2] = np.cos(ang)
    p["peT"] = np.ascontiguousarray(pe.T)  # [16, S]
    pf = np.zeros((128, 128), f32)
    for j in range(8):
        pf[:, j * 16 : (j + 1) * 16] = pe[128 * j : 128 * (j + 1), :]
    p["pe_fold"] = pf

    for pref in ("1", "2"):
        wq, wk, wv = inp[f"q{pref}_w"], inp[f"k{pref}_w"], inp[f"v{pref}_w"]
        Wq = np.zeros((48, 4, 32), f32)
        Wk = np.zeros((48, 4, 32), f32)
        for h in range(4):
            Wq[0:16, h, 0:4] = wq[4 * h : 4 * h + 4, :].T
            Wq[32:48, h, 4:8] = wq[4 * h : 4 * h + 4, :].T
            Wk[32:48, h, 0:4] = -wk[4 * h : 4 * h + 4, :].T
            Wk[0:16, h, 4:8] = wk[4 * h : 4 * h + 4, :].T
        p[f"Wq{pref}"] = Wq
        p[f"Wk{pref}"] = Wk
        Wv = np.zeros((17, 128), f32)
        for h in range(4):
            Wv[0:16, 32 * h : 32 * h + 4] = wv[4 * h : 4 * h + 4, :].T
            Wv[16, 32 * h + 4] = 1.0
        p[f"Wv{pref}"] = Wv

    p["sgnmask"] = np.sign(
        np.arange(TT, dtype=f32)[:, None] - np.arange(TT, dtype=f32)[None, :]
    ).astype(f32)
    sgnext = -np.ones((128, 512), f32)
    sgnext[:, 0:128] = p["sgnmask"]
    p["sgnext"] = sgnext
    sv = np.zeros((48, S), f32)
    sv[32:48, :] = -(np.arange(S, dtype=f32) / S)[None, :]
    p["svecneg"] = sv
    p["ones_row"] = np.ones((1, S), f32)
    sel = np.zeros((128, 20), f32)
    for h in range(4):
        for j in range(5):
            sel[32 * h + j, 5 * h + j] = 1.0
    p["selT"] = sel
    p["identity"] = np.eye(128, dtype=f32)
    p["identity16b"] = np.eye(16, dtype=f32)
    p["ones_col"] = np.ones((128, 1), f32)
    for nm in ("lna1", "ln1", "lna2", "ln2"):
        p[f"{nm}_g"] = np.broadcast_to(inp[f"{nm}_g"], (128, 16)).copy()
        p[f"{nm}_b"] = np.broadcast_to(inp[f"{nm}_b"], (128, 16)).copy()
    selE = np.zeros((128, 16), f32)
    for j in range(8):
        for e in range(16):
            selE[16 * j + e, e] = 1.0 / S
    p["selE"] = selE
    p["woutT"] = np.ascontiguousarray(inp["out_w"].T)
    p["bout"] = inp["out_b"].reshape(1, NCLS).astype(f32)

    x = inp["x"][:, 0, :]
    x8 = np.zeros((B, 8, S), f32)
    for k in range(8):
        x8[:, k, :] = x[:, k : k + 4 * S : 4][:, :S]
    p["x8"] = x8

    # which LN affine transforms are trivial (skip ops)
    p["_ln_trivial"] = {
        nm: bool(
            np.allclose(inp[f"{nm}_g"], 1.0) and np.allclose(inp[f"{nm}_b"], 0.0)
        )
        for nm in ("lna1", "ln1", "lna2", "ln2")
    }
    return p


# ------------------------------------------------------------- kernel build
_BUILD_CACHE = {}

CONST_SPECS = [
    # name, shape, dtype tag: False=f32, True=bf16, "r"=float32r
    ("w1T", (8, 8), True),
    ("b1", (8, 1), False),
    ("w2tap", (8, 8, 32), True),
    ("b2", (32, 1), False),
    ("w3T", (128, 8, 16), True),
    ("b3", (16, 1), False),
    ("peT", (16, S), True),
    ("pe_fold", (128, 128), False),
    ("Wq1", (48, 4, 32), True),
    ("Wk1", (48, 4, 32), True),
    ("Wv1", (17, 128), True),
    ("Wq2", (48, 4, 32), True),
    ("Wk2", (48, 4, 32), True),
    ("Wv2", (17, 128), True),
    ("sgnmask", (128, 128), False),
    ("sgnext", (128, 512), False),
    ("svecneg", (48, S), True),
    ("ones_row", (1, S), True),
    ("selT", (128, 20), False),
    ("identity", (128, 128), False),
    ("identity16b", (16, 16), True),
    ("ones_col", (128, 1), False),
    ("lna1_g", (128, 16), False),
    ("lna1_b", (128, 16), False),
    ("ln1_g", (128, 16), False),
    ("ln1_b", (128, 16), False),
    ("lna2_g", (128, 16), False),
    ("lna2_b", (128, 16), False),
    ("ln2_g", (128, 16), False),
    ("ln2_b", (128, 16), False),
    ("selE", (128, 16), False),
    ("woutT", (16, NCLS), False),
    ("bout", (1, NCLS), False),
]


def _brd(ap, count):
    """Append a broadcast (step 0) innermost free dim to an AP."""
    import concourse.bass as bass

    return bass.AP(tensor=ap.tensor, offset=ap.offset, ap=[*list(ap.ap), [0, count]])


def build_nc(ln_trivial, reps=1, cfg=None):
    import concourse.bass as bass
    import concourse.bacc as bacc
    import concourse.tile as tile
    from concourse import mybir

    if cfg is None:
        cfg = CFG
    n_dve = cfg["n_dve"]
    rsqrt_iters = cfg["rsqrt_iters"]

    f32 = mybir.dt.float32
    f32r = mybir.dt.float32r
    bf16 = mybir.dt.bfloat16
    i16 = mybir.dt.int16
    ALU = mybir.AluOpType
    AF = mybir.ActivationFunctionType

    # (t, hp) pairs per sh assigned to the DVE-Schraudolph exp, spread
    # across t so ScalarE and DVE run concurrently.
    def dve_pairs_for(sh):
        ts = [t for t in range(8) if t // 4 != sh]
        order = [
            (ts[0], 0), (ts[1], 1), (ts[2], 0), (ts[3], 1),
            (ts[0], 1), (ts[1], 0), (ts[2], 1), (ts[3], 0),
        ]
        return set(order[:n_dve])

    DVE_PAIRS = [dve_pairs_for(0), dve_pairs_for(1)]

    nc = bacc.Bacc(trn_type="TRN2", target_bir_lowering=False, debug=False)

    dram = {}
    dt_of = {False: f32, True: bf16, "r": f32r}
    for name, shape, isbf in CONST_SPECS:
        dram[name] = nc.dram_tensor(
            name, list(shape), dt_of[isbf], kind="ExternalInput"
        ).ap()
    dram["x8"] = nc.dram_tensor("x8", [NB, 8, S], bf16, kind="ExternalInput").ap()
    out_d = nc.dram_tensor("out", [NB, NCLS], f32, kind="ExternalOutput").ap()

    with tile.TileContext(nc) as tc:
        import contextlib

        ctx = contextlib.ExitStack()
        cpool = ctx.enter_context(tc.tile_pool(name="consts", bufs=1))
        perb = ctx.enter_context(tc.tile_pool(name="perb", bufs=NB))
        work = ctx.enter_context(tc.tile_pool(name="work", bufs=4))
        wexp = ctx.enter_context(tc.tile_pool(name="wexp", bufs=6))
        small = ctx.enter_context(tc.tile_pool(name="small", bufs=8))
        pp_s = ctx.enter_context(tc.tile_pool(name="pp_s", bufs=3, space="PSUM"))
        pp_av = ctx.enter_context(tc.tile_pool(name="pp_av", bufs=1, space="PSUM"))
        pp_sm = ctx.enter_context(tc.tile_pool(name="pp_sm", bufs=1, space="PSUM"))

        # ---- constants: critical subset DMA'd now, the rest later
        EARLY = {
            "w1T", "b1", "w2tap", "b2", "w3T", "b3", "identity",
            "identity16b", "peT",
            "ones_row", "svecneg", "Wq1", "Wk1", "Wv1",
            "sgnmask", "sgnext", "selT",
        }
        C = {}
        _late_consts = []
        # conv-critical consts first so batch 0's conv chain starts ASAP;
        # remaining EARLY consts queue behind it, LATE ones after conv(2).
        _prio = ["w1T", "b1", "w2tap", "b2", "w3T", "b3", "identity"]
        _order = _prio + [
            n for n, _, _ in CONST_SPECS if n not in _prio
        ]
        _by_name = {n: (n, s, d) for n, s, d in CONST_SPECS}
        for name in _order:
            name, shape, isbf = _by_name[name]
            t = cpool.tile(list(shape), dt_of[isbf], name=f"c_{name}")
            C[name] = t
            if name not in EARLY:
                _late_consts.append(name)
        for name in _prio:
            nc.sync.dma_start(out=C[name], in_=dram[name])

        def load_late_consts():
            for name in _late_consts:
                nc.sync.dma_start(out=C[name], in_=dram[name])

        h4_b = []        # [16, S] f32 per batch (x_srcT)
        xsrc_fold_b = [] # [128, 128] f32
        att_fold_b = [None] * NB

        eps_sb = cpool.tile([128, 1], f32, name="eps_sb")
        nc.vector.memset(eps_sb, EPS)
        magic_sb = cpool.tile([128, 1], mybir.dt.int32, name="magic_sb")
        nc.vector.memset(magic_sb, 0x5F3759DF)

        def ln_fold(src, nm, extra_add=None):
            return ln_fold_multi([(src, nm, extra_add)])[0]

        def ln_fold_multi(items):
            """items: list of (src, nm, extra_add). Ops are emitted
            interleaved across items so independent chains pipeline on DVE."""
            outs = [None] * len(items)
            st = [dict() for _ in items]
            for i, (src, nm, extra_add) in enumerate(items):
                _ln_stage0(st[i], src, f"{nm}{i}", extra_add)
            for step in range(1, 12):
                for i, (src, nm, extra_add) in enumerate(items):
                    _ln_step(st[i], step, f"{items[i][1]}{i}")
            for i, (src, nm, extra_add) in enumerate(items):
                outs[i] = _ln_final(st[i], f"{nm}{i}", nm)
            return outs

        def _ln_stage0(S_, src, nm, extra_add):
            """LN over e-groups of 16 in folded [128, (j,16)] layout.
            src: [128,128] f32 SBUF tile. Returns new [128,128] f32 tile.
            extra_add: optional [128,128] tile added BEFORE the LN (residual)."""
            if extra_add is not None:
                tmp = small.tile([128, 128], f32, name=f"res_{nm}", tag="lnres")
                nc.vector.tensor_add(tmp, src, extra_add)
                src = tmp
            S_["s3d"] = src.rearrange("p (j e) -> p j e", e=16)

        def _ln_step(S_, step, nm):
            i32 = mybir.dt.int32
            if step == 1:
                S_["sums"] = small.tile([128, 8], f32, name=f"sums_{nm}", tag="lnsum")
                nc.vector.tensor_reduce(
                    out=S_["sums"], in_=S_["s3d"], axis=mybir.AxisListType.X,
                    op=ALU.add,
                )
            elif step == 2:
                S_["negmean"] = small.tile([128, 8], f32, name=f"nm_{nm}", tag="lnnm")
                nc.vector.tensor_scalar_mul(S_["negmean"], S_["sums"], -1.0 / 16.0)
            elif step == 3:
                S_["cen"] = small.tile([128, 8, 16], f32, name=f"cen_{nm}", tag="lncen")
                nc.vector.tensor_tensor(
                    out=S_["cen"], in0=S_["s3d"], in1=_brd(S_["negmean"], 16),
                    op=ALU.add,
                )
            elif step == 4:
                S_["sq"] = small.tile([128, 8, 16], f32, name=f"sq_{nm}", tag="lnsq")
                nc.vector.tensor_mul(S_["sq"], S_["cen"], S_["cen"])
            elif step == 5:
                var = small.tile([128, 8], f32, name=f"var_{nm}", tag="lnvar")
                nc.vector.tensor_reduce(
                    out=var, in_=S_["sq"], axis=mybir.AxisListType.X, op=ALU.add
                )
                S_["var"] = var
            elif step == 6:
                v16 = small.tile([128, 8], f32, name=f"v16_{nm}", tag="lnv16")
                nc.vector.tensor_scalar(
                    out=v16, in0=S_["var"], scalar1=1.0 / 16.0, scalar2=EPS,
                    op0=ALU.mult, op1=ALU.add,
                )
                S_["v16"] = v16
                ish = small.tile([128, 8], i32, name=f"ish_{nm}", tag="lnish")
                nc.vector.tensor_scalar(
                    out=ish, in0=v16.bitcast(i32), scalar1=1, scalar2=None,
                    op0=ALU.arith_shift_right,
                )
                S_["ish"] = ish
            elif step == 7:
                y = small.tile([128, 8], i32, name=f"y0_{nm}", tag="lny0")
                nc.vector.tensor_tensor(
                    out=y, in0=_brd(magic_sb, 8), in1=S_["ish"], op=ALU.subtract
                )
                S_["y"] = y.bitcast(f32)
            elif step in (8, 9):
                it = step - 8
                if it >= rsqrt_iters:
                    return
                y = S_["y"]
                t1 = small.tile([128, 8], f32, name=f"nt1_{nm}_{it}", tag="lnnt1")
                nc.vector.tensor_mul(t1, y, y)
                t2 = small.tile([128, 8], f32, name=f"nt2_{nm}_{it}", tag="lnnt2")
                nc.vector.tensor_mul(t2, t1, S_["v16"])
                t3 = small.tile([128, 8], f32, name=f"nt3_{nm}_{it}", tag="lnnt3")
                nc.vector.tensor_scalar(
                    out=t3, in0=t2, scalar1=-0.5, scalar2=1.5,
                    op0=ALU.mult, op1=ALU.add,
                )
                yn = small.tile([128, 8], f32, name=f"yn_{nm}_{it}", tag="lnyn")
                nc.vector.tensor_mul(yn, y, t3)
                S_["y"] = yn
            elif step == 10:
                dst = small.tile([128, 8, 16], f32, name=f"ln_{nm}", tag="lnout")
                nc.vector.tensor_tensor(
                    out=dst, in0=S_["cen"], in1=_brd(S_["y"], 16), op=ALU.mult
                )
                S_["dst"] = dst

        def _ln_final(S_, nm, base_nm):
            dst = S_["dst"]
            dst2 = dst.rearrange("p j e -> p (j e)")
            base = base_nm[:-2] if base_nm.endswith("_l") else base_nm
            if not ln_trivial[base]:
                g3 = C[f"{base}_g"].rearrange("p e -> p 1 e")
                b3 = C[f"{base}_b"].rearrange("p e -> p 1 e")
                dstg = small.tile([128, 8, 16], f32, name=f"lng_{nm}", tag="lnoutg")
                nc.vector.tensor_tensor(
                    out=dstg,
                    in0=dst,
                    in1=bass.AP(
                        tensor=g3.tensor, offset=g3.offset,
                        ap=[g3.ap[0], [0, 8], g3.ap[2]],
                    ),
                    op=ALU.mult,
                )
                dstb = small.tile([128, 8, 16], f32, name=f"lnb_{nm}", tag="lnoutb")
                nc.vector.tensor_tensor(
                    out=dstb,
                    in0=dstg,
                    in1=bass.AP(
                        tensor=b3.tensor, offset=b3.offset,
                        ap=[b3.ap[0], [0, 8], b3.ap[2]],
                    ),
                    op=ALU.add,
                )
                dst2 = dstb.rearrange("p j e -> p (j e)")
            return dst2

        # ================= pipeline =================
        h4_b = {}
        xsrc_fold_b = {}
        P = {}  # (li, b) -> dict of prep tiles
        O = {}  # (li, b) -> o_sb/o_ps refs

        x8_tiles = {}

        def load_x8(b):
            x8 = work.tile([8, S], bf16, name=f"x8_{b}", tag="x8")
            nc.sync.dma_start(out=x8, in_=dram["x8"][b])
            x8_tiles[b] = x8

        def conv_frontend(b):
            x8 = x8_tiles[b]
            h1p = work.tile([8, S + 8], bf16, name=f"h1p_{b}", tag="h1p")
            nc.vector.memset(h1p[:, 0:3], 0.0)
            nc.vector.memset(h1p[:, 3 + S :], 0.0)
            for c in range(2):
                ps = pp_sm.tile([8, 512], f32, name=f"h1ps_{b}_{c}", tag="psmall")
                nc.tensor.matmul(
                    ps, C["w1T"], x8[:, 512 * c : 512 * (c + 1)],
                    start=True, stop=True,
                )
                nc.vector.tensor_scalar(
                    out=h1p[:, 3 + 512 * c : 3 + 512 * (c + 1)], in0=ps,
                    scalar1=C["b1"], scalar2=0.0, op0=ALU.add, op1=ALU.max,
                )
            h2p = work.tile([32, S + 36], bf16, name=f"h2p_{b}", tag="h2p")
            nc.vector.memset(h2p[:, 0:15], 0.0)
            nc.vector.memset(h2p[:, 15 + S :], 0.0)
            for c in range(2):
                ps = pp_sm.tile([32, 512], f32, name=f"h2ps_{b}_{c}", tag="psmall")
                for k in range(8):
                    nc.tensor.matmul(
                        ps, C["w2tap"][:, k, :],
                        h1p[:, k + 512 * c : k + 512 * c + 512],
                        start=(k == 0), stop=(k == 7),
                    )
                nc.vector.tensor_scalar(
                    out=h2p[:, 15 + 512 * c : 15 + 512 * (c + 1)], in0=ps,
                    scalar1=C["b2"], scalar2=0.0, op0=ALU.add, op1=ALU.max,
                )
            h2im = work.tile([128, S + 36], bf16, name=f"h2im_{b}", tag="h2im")
            for kk in range(4):
                nc.sync.dma_start(
                    out=h2im[32 * kk : 32 * kk + 32, 0 : S + 32],
                    in_=h2p[:, kk : kk + S + 32],
                )
            h4 = perb.tile([16, S], bf16, name=f"h4_{b}", tag="h4")
            for c in range(2):
                ps = pp_sm.tile([16, 512], f32, name=f"h3ps_{b}_{c}", tag="psmall")
                for g in range(8):
                    nc.tensor.matmul(
                        ps, C["w3T"][:, g, :],
                        h2im[:, 4 * g + 512 * c : 4 * g + 512 * c + 512],
                        start=(g == 0), stop=(g == 7),
                    )
                nc.vector.tensor_scalar(
                    out=h4[:, 512 * c : 512 * (c + 1)], in0=ps,
                    scalar1=C["b3"], scalar2=0.0, op0=ALU.add, op1=ALU.max,
                )
            h4_b[b] = h4
            xs_ps = pp_sm.tile([128, 128], bf16, name=f"xsps_{b}", tag="psmall")
            for j in range(8):
                nc.tensor.transpose(
                    xs_ps[:, 16 * j : 16 * j + 16],
                    h4[:, 128 * j : 128 * (j + 1)],
                    C["identity16b"],
                )
            xsf = perb.tile([128, 128], f32, name=f"xsf_{b}", tag="xsf")
            nc.vector.tensor_copy(xsf, xs_ps)
            xsrc_fold_b[b] = xsf

        def attn_prep(li, pref, b):
            # xx: rows 0-15 xT, 16 ones, 17-31 zero, 32-47 xhatneg=-(s/n)*xT
            xx = work.tile([48, S], bf16, name=f"xx_{li}_{b}", tag="xx")
            nc.gpsimd.memset(xx[0:48, :], 0.0)
            if li == 0:
                nc.vector.tensor_add(xx[0:16, :], h4_b[b], C["peT"])
            else:
                x2f = small.tile([128, 128], f32, name=f"x2f_{b}", tag="x2f")
                nc.vector.tensor_add(x2f, att_fold_b[b], C["pe_fold"])
                for half in range(2):
                    t2 = pp_sm.tile(
                        [16, 512], f32, name=f"t2_{b}_{half}", tag="psmall"
                    )
                    for j in range(4):
                        jj = 4 * half + j
                        nc.tensor.transpose(
                            t2[:, 128 * j : 128 * (j + 1)],
                            x2f[:, 16 * jj : 16 * jj + 16],
                            C["identity"],
                        )
                    nc.vector.tensor_copy(
                        xx[0:16, 512 * half : 512 * (half + 1)], t2
                    )
            nc.gpsimd.dma_start(out=xx[16:17, :], in_=C["ones_row"])
            nc.gpsimd.dma_start(out=xx[32:48, :], in_=xx[0:16, :])
            nc.vector.tensor_mul(
                xx[32:48, :], xx[32:48, :], C["svecneg"][32:48, :]
            )
            qT = work.tile([128, S], bf16, name=f"qT_{li}_{b}", tag="qT")
            kT = work.tile([128, S], bf16, name=f"kT_{li}_{b}", tag="kT")
            for dst, wname in ((qT, f"Wq{pref}"), (kT, f"Wk{pref}")):
                for c in range(2):
                    ps = pp_sm.tile(
                        [128, 512], f32, name=f"qk_{li}_{b}_{c}", tag="psmall"
                    )
                    for h in range(4):
                        nc.tensor.matmul(
                            ps[32 * h : 32 * h + 32, :],
                            C[wname][:, h, :],
                            xx[:, 512 * c : 512 * (c + 1)],
                            start=True, stop=True, tile_position=(0, 32 * h),
                        )
                    nc.vector.tensor_copy(dst[:, 512 * c : 512 * (c + 1)], ps)
            v_sb = work.tile([128, 8, 128], bf16, name=f"v_{li}_{b}", tag="v")
            for g in range(2):
                vps4 = pp_sm.tile(
                    [128, 4, 128], f32, name=f"vps_{li}_{b}_{g}", tag="psmall"
                )
                for tt in range(4):
                    t = 4 * g + tt
                    nc.tensor.matmul(
                        vps4[:, tt, :], xx[0:17, 128 * t : 128 * (t + 1)],
                        C[f"Wv{pref}"], start=True, stop=True,
                    )
                nc.vector.tensor_copy(v_sb[:, 4 * g : 4 * g + 4, :], vps4)
            P[(li, b)] = dict(qT=qT, kT=kT, v_sb=v_sb)

        def attn_core_pair(li, pref, pair):
            o_sb = {}
            for b in pair:
                o_sb[b] = work.tile([128, S], f32, name=f"osb_{li}_{b}", tag="osb")
            for sh in range(2):
                o_ps = {}
                for b in pair:
                    o_ps[b] = pp_av.tile(
                        [128, 512], f32, name=f"ops_{li}_{b}_{sh}", tag="ops"
                    )
                for t in range(8):
                    for hp in range(2):
                        for b in pair:
                            qT, kT, v_sb = (
                                P[(li, b)]["qT"], P[(li, b)]["kT"], P[(li, b)]["v_sb"]
                            )
                            sps = pp_s.tile(
                                [128, 2, 512], f32,
                                name=f"sps_{li}_{b}_{sh}_{t}_{hp}", tag="sps",
                            )
                            for hh in range(2):
                                h = 2 * hp + hh
                                nc.tensor.matmul(
                                    sps[:, hh, :],
                                    kT[32 * h : 32 * h + 8, 128 * t : 128 * (t + 1)],
                                    qT[32 * h : 32 * h + 8, 512 * sh : 512 * (sh + 1)],
                                    start=True, stop=True,
                                    tile_position=(32 * h, 0),
                                )
                            aT = wexp.tile(
                                [128, 2, 512], bf16,
                                name=f"aT_{li}_{b}_{sh}_{t}_{hp}", tag="aT",
                            )
                            if t // 4 != sh and (t, hp) in DVE_PAIRS[sh]:
                                # DVE Schraudolph exp-bits (offloads ScalarE)
                                sgn = 1.0 if t // 4 > sh else -1.0
                                nc.vector.tensor_scalar(
                                    out=aT.bitcast(i16), in0=sps,
                                    scalar1=sgn * SCH_A, scalar2=SCH_B,
                                    op0=ALU.mult, op1=ALU.add,
                                )
                            elif t // 4 == sh:
                                lc = 128 * t - 512 * sh
                                if lc >= 256:
                                    # cheap extended sign fix -> 1 exp call
                                    sgn = C["sgnext"]
                                    nc.vector.tensor_tensor(
                                        out=sps[:, :, lc:512],
                                        in0=sps[:, :, lc:512],
                                        in1=bass.AP(
                                            tensor=sgn.tensor, offset=sgn.offset,
                                            ap=[sgn.ap[0], [0, 2], [1, 512 - lc]],
                                        ),
                                        op=ALU.mult,
                                    )
                                    nc.scalar.activation(
                                        out=aT, in_=sps, func=AF.Exp, scale=CSCALE
                                    )
                                else:
                                    # classic diag fix + split exp
                                    sgn = C["sgnmask"]
                                    nc.vector.tensor_tensor(
                                        out=sps[:, :, lc : lc + 128],
                                        in0=sps[:, :, lc : lc + 128],
                                        in1=bass.AP(
                                            tensor=sgn.tensor, offset=sgn.offset,
                                            ap=[sgn.ap[0], [0, 2], sgn.ap[1]],
                                        ),
                                        op=ALU.mult,
                                    )
                                    bnd = lc + 128
                                    nc.scalar.activation(
                                        out=aT[:, :, 0:bnd], in_=sps[:, :, 0:bnd],
                                        func=AF.Exp, scale=CSCALE,
                                    )
                                    nc.scalar.activation(
                                        out=aT[:, :, bnd:512], in_=sps[:, :, bnd:512],
                                        func=AF.Exp, scale=-CSCALE,
                                    )
                            else:
                                scl = CSCALE if t // 4 > sh else -CSCALE
                                nc.scalar.activation(
                                    out=aT, in_=sps, func=AF.Exp, scale=scl
                                )
                            for hh in range(2):
                                h = 2 * hp + hh
                                nc.tensor.matmul(
                                    o_ps[b][32 * h : 32 * h + 32, :],
                                    v_sb[:, t, 32 * h : 32 * h + 32],
                                    aT[:, hh, :],
                                    start=(t == 0), stop=(t == 7),
                                    tile_position=(0, 32 * h),
                                    skip_group_check=True,
                                )
                for b in pair:
                    nc.vector.tensor_copy(
                        o_sb[b][:, 512 * sh : 512 * (sh + 1)], o_ps[b]
                    )
            for b in pair:
                O[(li, b)] = o_sb[b]

        def attn_epilogue(li, pref, b):
            o_sb = O[(li, b)]
            T_ps = pp_sm.tile([128, 8, 20], f32, name=f"Tps_{li}_{b}", tag="psmall")
            for j in range(8):
                nc.tensor.matmul(
                    T_ps[:, j, :], o_sb[:, 128 * j : 128 * (j + 1)], C["selT"],
                    start=True, stop=True,
                )
            T4 = T_ps.rearrange("p j (h c) -> p j h c", h=4)
            r_sb = small.tile([128, 8, 4], f32, name=f"r_{li}_{b}", tag="recip")
            nc.vector.reciprocal(out=r_sb, in_=T4[:, :, :, 4])
            araw = small.tile([128, 8, 4, 4], f32, name=f"araw_{li}_{b}", tag="araw")
            nc.vector.tensor_tensor(
                out=araw, in0=T4[:, :, :, 0:4], in1=_brd(r_sb, 4), op=ALU.mult
            )
            araw2 = araw.rearrange("p j h c -> p (j h c)")
            return araw2

        def attn_ln_stage(li, pref, pair, araws):
            anorms = ln_fold_multi(
                [(a, f"lna{pref}_l", None) for a in araws]
            )
            if li == 0:
                att1s = ln_fold_multi(
                    [
                        (anorms[i], "ln1_l", xsrc_fold_b[b])
                        for i, b in enumerate(pair)
                    ]
                )
                for i, b in enumerate(pair):
                    att1p = perb.tile([128, 128], f32, name=f"att1_{b}", tag="attf")
                    nc.vector.tensor_copy(att1p, att1s[i])
                    att_fold_b[b] = att1p
                return
            att2s = ln_fold_multi([(a, "ln2_l", None) for a in anorms])
            for i, b in enumerate(pair):
                att2 = att2s[i]
                att2p = small.tile([128, 128], f32, name=f"att2_{b}", tag="att2")
                nc.vector.tensor_copy(att2p, att2)
                cs_ps = pp_sm.tile([128, 1], f32, name=f"cs_{b}", tag="psmall")
                nc.tensor.matmul(cs_ps, att2p, C["ones_col"], start=True, stop=True)
                cs_sb = small.tile([128, 1], f32, name=f"cssb_{b}", tag="cssb")
                nc.vector.tensor_copy(cs_sb, cs_ps)
                pl_ps = pp_sm.tile([16, 1], f32, name=f"pl_{b}", tag="psmall")
                nc.tensor.matmul(pl_ps, C["selE"], cs_sb, start=True, stop=True)
                pl_sb = small.tile([16, 1], f32, name=f"plsb_{b}", tag="plsb")
                nc.vector.tensor_copy(pl_sb, pl_ps)
                o10_ps = pp_sm.tile([1, NCLS], f32, name=f"o10_{b}", tag="psmall")
                nc.tensor.matmul(o10_ps, pl_sb, C["woutT"], start=True, stop=True)
                o10 = small.tile([1, NCLS], f32, name=f"o10sb_{b}", tag="o10")
                nc.vector.tensor_add(o10, o10_ps, C["bout"])
                nc.sync.dma_start(out=out_d[b], in_=o10)

        # ---- software-pipelined emission
        pairs = [(0, 1), (2, 3)]

        def _epi(li, pref, pair):
            araws = [attn_epilogue(li, pref, b) for b in pair]
            attn_ln_stage(li, pref, pair, araws)

        def _emit_pipeline(first):
            load_x8(0)
            for name in sorted(EARLY - set(_prio)):
                nc.sync.dma_start(out=C[name], in_=dram[name])
            for b in range(1, NB):
                load_x8(b)
            conv_frontend(0)
            attn_prep(0, "1", 0)
            attn_core_pair(0, "1", (0,))
            conv_frontend(1)
            attn_prep(0, "1", 1)
            attn_core_pair(0, "1", (1,))
            conv_frontend(2)
            attn_prep(0, "1", 2)
            if first:
                load_late_consts()
            _epi(0, "1", (0,))
            attn_core_pair(0, "1", (2,))
            conv_frontend(3)
            attn_prep(0, "1", 3)
            _epi(0, "1", (1,))
            attn_core_pair(0, "1", (3,))
            attn_prep(1, "2", 0)
            _epi(0, "1", (2,))
            attn_core_pair(1, "2", (0,))
            attn_prep(1, "2", 1)
            _epi(0, "1", (3,))
            attn_core_pair(1, "2", (1,))
            attn_prep(1, "2", 2)
            _epi(1, "2", (0,))
            attn_core_pair(1, "2", (2,))
            attn_prep(1, "2", 3)
            _epi(1, "2", (1,))
            _epi(1, "2", (2,))
            attn_core_pair(1, "2", (3,))
            _epi(1, "2", (3,))

        for _rep in range(reps):
            _emit_pipeline(_rep == 0)
        ctx.close()

    nc.compile()
    return nc


def _get_nc(ln_trivial_key, reps=1, cfg=None):
    if cfg is None:
        cfg = CFG
    key = (
        tuple(sorted(ln_trivial_key.items())),
        reps,
        tuple(sorted(cfg.items())),
    )
    if key not in _BUILD_CACHE:
        _BUILD_CACHE[key] = build_nc(ln_trivial_key, reps, cfg)
    return _BUILD_CACHE[key]


# ------------------------------------------------------------------ runner
def _run(inputs, trace=False, **kw):
    import ml_dtypes
    from concourse import bass_utils

    p = host_prep(inputs)
    nc = _get_nc(p["_ln_trivial"])

    base = {}
    for name, shape, isbf in CONST_SPECS:
        a = p[name].astype(ml_dtypes.bfloat16 if isbf is True else np.float32)
        assert a.shape == shape, (name, a.shape, shape)
        base[name] = a
    in_maps = []
    for c in range(NCORES):
        m = dict(base)
        m["x8"] = np.ascontiguousarray(p["x8"][NB * c : NB * (c + 1)]).astype(ml_dtypes.bfloat16)
        in_maps.append(m)

    res = bass_utils.run_bass_kernel_spmd(
        nc, in_maps, core_ids=list(range(NCORES)), trace=trace, **kw
    )
    out = np.concatenate(
        [res.results[c]["out"] for c in range(NCORES)], axis=0
    ).astype(np.float32)
    return out, res


def kernel(**inputs) -> np.ndarray:
    out, _ = _run(inputs, trace=False)
    return out


if __name__ == "__main__":
    # standalone smoke test (requires reference.py + cached inputs)
    import os

    if os.path.exists("/tmp/inputs.npz"):
        inputs = dict(np.load("/tmp/inputs.npz"))
    else:
        import reference

        inputs = {k: np.asarray(v) for k, v in reference.setup_inputs().items()}
    out = kernel(**inputs)
    print("out shape:", out.shape, "out[0]:", out[0])

